# revision 31
# baseline (speedup 1.0000x reference)
"""Trainium2 Bass kernel for nn_Classifier (capsule-style conv + routing).

Math (validated against the jax reference, CPU-emulated to 3.4e-3 rel):
  W = conv_w[:,0,:]                                   # [16, 640]
  U[b,m,o]   = relu(sum_t x[b,m,t] W[t,o] + conv_b[o])          (conv as matmul, K=16)
  usum[b,o]  = sum_m U[b,m,o]
  logits     = (usum . U)/4 per class block -> stable softmax over m -> C
  Cb         = C + B_bias[k,m]
  S[b,k,:]   = sum_m Cb[b,k,m] U[b,m,k*64:+64]
  out[b,k]   = n2/(n2+1) with n2 = |S|^2

v3 design (vs v2 at ~85us):
  - ALL matmul operands fp16 (1 pass/row on PE; v2's fp32 routing matmuls
    ran 4 cycles/row).  fp16 (10 mantissa bits) instead of bf16 keeps the
    logit error ~4x smaller; CPU-emulated end-to-end rel err 3.4e-3.
  - U computed ONCE (oi orientation: [o-part, m-free]); relu+eviction
    fused with usum accum_out.  The io orientation ([m-part, o-free],
    needed by the S matmul) is produced by DMA xbar transposes
    (SBUF->SBUF, idle engine) instead of a second conv + a second
    PSUM->SBUF eviction pass (v2 paid ~25us of engine-sum for that).
  - oi layout (c,m), io layout (c,q,o) make every per-(b,c) transpose a
    contiguous [128,512]->[128,512] dma_start_transpose call; the S
    matmul reads io chunks through a 3D access pattern.
  - evictions alternate DVE/ACT (roughly 741ns vs 896ns per tile).
  - Cb transposes output fp16 PSUM (legal for transpose-mode matmuls)
    so their eviction runs in the DVE 2x packed mode.
"""

import numpy as np

import concourse.bass as bass
import concourse.mybir as mybir
import concourse.tile as tile
from concourse import bacc
from concourse.bass_utils import run_bass_kernel_spmd

F32 = mybir.dt.float32
F16 = mybir.dt.float16

B_FULL = 64
N = 512          # num timecaps (routing dim m)
DT = 16          # dim timecaps (conv contraction)
K = 10           # classes
D = 64           # dim classes
NO = K * D       # 640 conv output channels
NCORES = 8
BPC = B_FULL // NCORES   # 8 batches per core
N_WARM = 2               # fp16 N=512 warmup matmuls (~3.4us cold -> HAM warm)

NP_F16 = mybir.dt.np(F16)

AF = mybir.ActivationFunctionType
OP = mybir.AluOpType


def _build_program():
    nc = bacc.Bacc("TRN2", target_bir_lowering=False)
    xt_in = nc.declare_dram_parameter("xt", [2, 128, N], F16, isOutput=False)
    w_in = nc.declare_dram_parameter("w", [128, NO], F16, isOutput=False)
    bb_in = nc.declare_dram_parameter("bb", [128, N], F16, isOutput=False)
    id16_in = nc.declare_dram_parameter("id16", [128, 128], F16, isOutput=False)
    gm_in = nc.declare_dram_parameter("gm", [128, 5 * 32], F16, isOutput=False)
    sm_in = nc.declare_dram_parameter("sm", [128, NO], F16, isOutput=False)
    out_d = nc.declare_dram_parameter("out", [BPC, K], F32, isOutput=True)

    with tile.TileContext(nc) as tc:
        with tc.tile_pool(name="const", bufs=1) as cpool:
            w_s = cpool.tile([128, NO], F16, name="w_s", tag="w_s")
            xT = [cpool.tile([128, N], F16, name=f"xT{g}", tag=f"xT{g}")
                  for g in range(2)]
            gmask = cpool.tile([128, 5 * 32], F16, name="gmask", tag="gmask")
            bb_s = cpool.tile([128, N], F16, name="bb_s", tag="bb_s")
            ident16 = cpool.tile([128, 128], F16, name="ident16", tag="ident16")
            smask = cpool.tile([128, NO], F16, name="smask", tag="smask")

            # DMA order = priority order: warmup operand first, then conv
            # operands, then late-phase constants.
            nc.sync.dma_start(w_s[:], w_in[:, :])
            for g in range(2):
                nc.sync.dma_start(xT[g][:], xt_in[g])
            nc.scalar.dma_start(gmask[:], gm_in[:, :])
            nc.scalar.dma_start(bb_s[:], bb_in[:, :])
            nc.gpsimd.dma_start(ident16[:], id16_in[:, :])
            nc.gpsimd.dma_start(smask[:], sm_in[:, :])

            # ---- HAM warmup: fp16 N=512 matmuls on the first-arrived w ----
            with tc.tile_pool(name="ps_warm", bufs=1, space="PSUM") as pw:
                ps_w = pw.tile([128, N], F32, name="ps_w", tag="ps_w")
                for r in range(N_WARM):
                    nc.tensor.matmul(
                        ps_w[:], w_s[0:128, 0:128], w_s[0:128, 0:N],
                        start=(r == 0), stop=(r == N_WARM - 1),
                    )

            with tc.tile_pool(name="data", bufs=1) as dpool:
                yr_oi = [dpool.tile([128, 5 * N], F16, name=f"yr_oi{b}",
                                    tag=f"yr_oi{b}") for b in range(BPC)]
                # io layout: one tile [128, (b, q, 640)] so multi-batch
                # eviction dests are single strided APs
                yr_io_all = dpool.tile([128, BPC * 4 * NO], F16,
                                       name="yr_io_all", tag="yr_io_all")
                yr_io = [yr_io_all[:, b * 4 * NO:(b + 1) * 4 * NO]
                         for b in range(BPC)]
                usum = dpool.tile([128, 5 * BPC], F32, name="usum", tag="usum")
                usum16 = [dpool.tile([128, 5], F16, name=f"usum16_{b}",
                                     tag=f"usum16_{b}") for b in range(BPC)]
                gmat = [dpool.tile([128, 5 * 32], F16, name=f"gmat{b}",
                                   tag=f"gmat{b}") for b in range(BPC)]
                exp_sb = [dpool.tile([128, N], F16, name=f"exp{g}", tag=f"exp{g}")
                          for g in range(2)]
                negmax = [dpool.tile([128, 1], F32, name=f"nm{g}", tag=f"nm{g}")
                          for g in range(2)]
                zsum = [dpool.tile([128, 1], F32, name=f"z{g}", tag=f"z{g}")
                        for g in range(2)]
                rz = [dpool.tile([128, 1], F32, name=f"rz{g}", tag=f"rz{g}")
                      for g in range(2)]
                cb_sb = [dpool.tile([128, N], F16, name=f"cb{g}", tag=f"cb{g}")
                         for g in range(2)]
                ebt_sb = [dpool.tile([128, N], F16, name=f"ebt{g}", tag=f"ebt{g}")
                          for g in range(2)]
                s_sb = [dpool.tile([128, NO], F16, name=f"s{g}", tag=f"s{g}")
                        for g in range(2)]
                n2 = dpool.tile([128, 2], F32, name="n2", tag="n2")
                n216 = dpool.tile([128, 2], F16, name="n216", tag="n216")
                t_c = dpool.tile([2, 128], F32, name="t_c", tag="t_c")
                t_d = dpool.tile([2, 128], F32, name="t_d", tag="t_d")
                outt = dpool.tile([2, 128], F32, name="outt", tag="outt")

                evict_idx = 0

                def evict(dst, src, acc=None):
                    nonlocal evict_idx
                    if evict_idx % 2 == 0:
                        if acc is not None:
                            nc.vector.tensor_scalar(
                                out=dst, in0=src, scalar1=0.0, scalar2=0.0,
                                op0=OP.max, op1=OP.add, accum_out=acc)
                        else:
                            nc.vector.tensor_scalar(
                                out=dst, in0=src, scalar1=0.0, scalar2=None,
                                op0=OP.max)
                    else:
                        nc.scalar.activation(
                            out=dst, in_=src, func=AF.Relu, accum_out=acc)
                    evict_idx += 1

                # PSUM: cv ring 4 + lg 1 + ebt 1 + s 2 = 8 banks
                with tc.tile_pool(name="ps_conv", bufs=4, space="PSUM") as pcv, \
                     tc.tile_pool(name="ps_lg", bufs=1, space="PSUM") as plg, \
                     tc.tile_pool(name="ps_ebt", bufs=1, space="PSUM") as peb, \
                     tc.tile_pool(name="ps_s", bufs=1, space="PSUM") as psp:
                    # ===== phase 1: conv-oi both groups, then routing =====
                    for g in range(2):
                        for c in range(5):
                            tiles = []
                            for j in range(4):
                                ps = pcv.tile([128, N], F32, name="ps_cv",
                                              tag="ps_cv")
                                nc.tensor.matmul(
                                    ps[:],
                                    w_s[32 * j:32 * j + DT + 1,
                                        c * 128:(c + 1) * 128],
                                    xT[g][32 * j:32 * j + DT + 1, :],
                                    start=True, stop=True,
                                    tile_position=(32 * j, 0),
                                )
                                tiles.append(ps)
                            for j in range(4):
                                b = 4 * g + j
                                evict(yr_oi[b][:, c * N:(c + 1) * N],
                                      tiles[j][:],
                                      usum[:, b * 5 + c:b * 5 + c + 1])

                    for g in range(2):
                        # gmat = gmask * usum (per batch, fp16 on gpsimd)
                        for j in range(4):
                            b = 4 * g + j
                            nc.gpsimd.tensor_copy(
                                usum16[b][:], usum[:, b * 5:b * 5 + 5])
                            nc.gpsimd.tensor_tensor(
                                out=gmat[b][:].rearrange(
                                    "p (c k) -> p c k", c=5),
                                in0=gmask[:].rearrange(
                                    "p (c k) -> p c k", c=5),
                                in1=usum16[b][:].unsqueeze(2).broadcast_to(
                                    [128, 5, 32]),
                                op=OP.mult,
                            )

                        # logits (col-tiled over j)
                        ps_lg = plg.tile([128, N], F32, name="ps_lg",
                                         tag="ps_lg")
                        for c in range(5):
                            for j in range(4):
                                b = 4 * g + j
                                nc.tensor.matmul(
                                    ps_lg[32 * j:32 * (j + 1), :],
                                    gmat[b][:, c * 32:(c + 1) * 32],
                                    yr_oi[b][:, c * N:(c + 1) * N],
                                    start=(c == 0), stop=(c == 4),
                                    tile_position=(0, 32 * j),
                                    skip_group_check=True,
                                )

                        # softmax -> Cb
                        nc.vector.tensor_reduce(
                            out=negmax[g][:], in_=ps_lg[:],
                            op=OP.max, axis=mybir.AxisListType.X, negate=True,
                        )
                        nc.scalar.activation(
                            out=exp_sb[g][:], in_=ps_lg[:], func=AF.Exp,
                            bias=negmax[g][:], scale=1.0,
                            accum_out=zsum[g][:],
                        )
                        nc.vector.reciprocal(rz[g][:], zsum[g][:])
                        nc.vector.scalar_tensor_tensor(
                            out=cb_sb[g][:], in0=exp_sb[g][:],
                            scalar=rz[g][:], in1=bb_s[:],
                            op0=OP.mult, op1=OP.add,
                        )

                        # Cb transpose (fp16 PSUM -> 2x eviction)
                        ps_eb = peb.tile([128, N], F16, name="ps_eb",
                                         tag="ps_eb")
                        for q in range(4):
                            nc.tensor.transpose(
                                ps_eb[:, q * 128:(q + 1) * 128],
                                cb_sb[g][:, q * 128:(q + 1) * 128],
                                ident16[:],
                            )
                        nc.vector.tensor_copy(ebt_sb[g][:], ps_eb[:])

                    # ===== phase 2 per g: conv-io, S, squash tail =====
                    # yr_io layout per batch: [128(m of chunk q), (q, 640 o)]
                    for g in range(2):
                        for q in range(4):
                            tiles = []
                            for j in range(4):
                                ps = pcv.tile([128, N], F32, name="ps_io",
                                              tag="ps_cv")
                                nc.tensor.matmul(
                                    ps[:],
                                    xT[g][32 * j:32 * j + DT + 1,
                                          q * 128:(q + 1) * 128],
                                    w_s[32 * j:32 * j + DT + 1, 0:512],
                                    start=True, stop=True,
                                    tile_position=(32 * j, 0),
                                )
                                tiles.append(ps)
                            for j in range(4):
                                b = 4 * g + j
                                evict(yr_io[b][:, q * NO:q * NO + 512],
                                      tiles[j][:])
                            # o-tail (cols 512:640) via PE transposes of the
                            # already-evicted yr_oi c=4 chunk: sequential
                            # writes into one fp16 PSUM bank, 2x eviction
                            ps_tl = peb.tile([128, N], F16, name="ps_tl",
                                             tag="ps_eb")
                            for j in range(4):
                                b = 4 * g + j
                                nc.tensor.transpose(
                                    ps_tl[:, j * 128:(j + 1) * 128],
                                    yr_oi[b][:, 4 * N + q * 128:
                                             4 * N + (q + 1) * 128],
                                    ident16[:],
                                )
                            evict(yr_io_all[:].rearrange(
                                      "p (b q o) -> p b q o", b=BPC, q=4)
                                  [:, 4 * g:4 * g + 4, q, 512:NO],
                                  ps_tl[:])
                    for g in range(2):
                        ps_s = psp.tile([128, NO], F32, name="ps_s", tag="ps_s")
                        for q in range(4):
                            # S matmuls for this q (col-tiled over j)
                            for j in range(4):
                                b = 4 * g + j
                                nc.tensor.matmul(
                                    ps_s[32 * j:32 * (j + 1), 0:512],
                                    ebt_sb[g][:, q * 128 + 32 * j:
                                              q * 128 + 32 * (j + 1)],
                                    yr_io[b][:, q * NO:q * NO + 512],
                                    start=(q == 0), stop=(q == 3),
                                    tile_position=(0, 32 * j),
                                    skip_group_check=True,
                                )
                                nc.tensor.matmul(
                                    ps_s[32 * j:32 * (j + 1), 512:NO],
                                    ebt_sb[g][:, q * 128 + 32 * j:
                                              q * 128 + 32 * (j + 1)],
                                    yr_io[b][:, q * NO + 512:q * NO + NO],
                                    start=(q == 0), stop=(q == 3),
                                    tile_position=(0, 32 * j),
                                    skip_group_check=True,
                                )

                        # masked squash tail
                        nc.vector.tensor_tensor(
                            out=s_sb[g][:], in0=ps_s[:], in1=smask[:],
                            op=OP.mult,
                        )
                        nc.vector.scalar_tensor_tensor(
                            out=s_sb[g][:], in0=s_sb[g][:],
                            scalar=1.0, in1=s_sb[g][:],
                            op0=OP.mult, op1=OP.mult,
                            accum_out=n2[:, g:g + 1],
                        )

                # transpose n2 -> [2(g), 128(32j+k)]; out = 1 - 1/(n2+1)
                with tc.tile_pool(name="ps_t2", bufs=1, space="PSUM") as pt2:
                    tr2 = pt2.tile([2, 128], F16, name="ps_tr2", tag="ps_tr2")
                    nc.gpsimd.tensor_copy(n216[:], n2[:])
                    nc.tensor.transpose(tr2[:], n216[:], ident16[:])
                    nc.vector.tensor_scalar(
                        out=t_c[:], in0=tr2[:], scalar1=1.0, scalar2=None,
                        op0=OP.add)
                    nc.vector.reciprocal(t_d[:], t_c[:])
                    nc.vector.tensor_scalar(
                        out=outt[:], in0=t_d[:], scalar1=-1.0, scalar2=1.0,
                        op0=OP.mult, op1=OP.add)
                    nc.sync.dma_start(
                        out_d.rearrange("(g j) k -> g j k", g=2),
                        outt[:].rearrange("g (j k) -> g j k", j=4)[:, :, 0:K],
                    )
    nc.compile()
    return nc


_PROGRAM_CACHE = None


def _get_program():
    global _PROGRAM_CACHE
    if _PROGRAM_CACHE is None:
        _PROGRAM_CACHE = _build_program()
    return _PROGRAM_CACHE


def make_in_maps(timecaps, conv_w, conv_b, B_bias):
    """Host-side prep: per-core transposed/padded operand arrays."""
    timecaps = np.ascontiguousarray(np.asarray(timecaps, dtype=np.float32))
    conv_w = np.asarray(conv_w, dtype=np.float32)
    conv_b = np.asarray(conv_b, dtype=np.float32)
    B_bias = np.asarray(B_bias, dtype=np.float32)

    w4 = np.zeros((128, NO), np.float32)
    bb = np.zeros((128, N), np.float32)
    gm = np.zeros((128, 5 * 32), np.float32)
    sm = np.zeros((128, NO), np.float32)
    for j in range(4):
        w4[32 * j:32 * j + DT] = conv_w[:, 0, :]
        w4[32 * j + DT] = conv_b
        bb[32 * j:32 * j + K] = B_bias[:, 0, :]
        sm_rows = sm[32 * j:32 * j + K]
        for k in range(K):
            sm_rows[k, D * k:D * (k + 1)] = 1.0
    for c in range(5):
        for p in range(128):
            gm[p, c * 32 + (c * 128 + p) // D] = 0.25
    ident = np.eye(128, dtype=np.float32)

    # xT per core: [2, 128, 512], rows 32j+t = x[4g+j, :, t], row 32j+16 = 1
    xs = timecaps.reshape(NCORES, 2, 4, N, DT).transpose(0, 1, 2, 4, 3)
    in_maps = []
    shared = {
        "w": w4.astype(NP_F16),
        "bb": bb.astype(NP_F16),
        "id16": ident.astype(NP_F16),
        "gm": gm.astype(NP_F16),
        "sm": sm.astype(NP_F16),
    }
    for core in range(NCORES):
        xt = np.zeros((2, 128, N), np.float32)
        for j in range(4):
            xt[:, 32 * j:32 * j + DT] = xs[core, :, j]
            xt[:, 32 * j + DT] = 1.0
        in_maps.append({"xt": xt.astype(NP_F16), **shared})
    return in_maps


def kernel(timecaps, conv_w, conv_b, B_bias):
    nc = _get_program()
    in_maps = make_in_maps(timecaps, conv_w, conv_b, B_bias)
    res = run_bass_kernel_spmd(nc, in_maps, list(range(NCORES)))
    out = np.concatenate([res.results[i]["out"] for i in range(NCORES)], axis=0)
    return out.reshape(B_FULL, K, 1).astype(np.float32)


if __name__ == "__main__":
    rng = np.random.default_rng(0)
    ins = {
        "timecaps": rng.standard_normal((B_FULL, N, DT), dtype=np.float32),
        "conv_w": (rng.standard_normal((DT, 1, NO), dtype=np.float32) * 0.05),
        "conv_b": np.zeros((NO,), dtype=np.float32),
        "B_bias": (rng.standard_normal((K, 1, N), dtype=np.float32) * 0.05),
    }
    print(kernel(**ins)[:2, :, 0])


# revision 33
# speedup vs baseline: 1.0365x; 1.0365x over previous
"""Trainium2 Bass kernel for nn_Classifier (capsule-style conv + routing).

Math (validated against the jax reference; HW rel err 3.5e-3):
  W = conv_w[:,0,:]                                   # [16, 640]
  U[b,m,o]   = relu(sum_t x[b,m,t] W[t,o] + conv_b[o])    (conv as matmul)
  usum[b,o]  = sum_m U[b,m,o]
  logits     = (usum . U)/4 per class block -> stable softmax over m -> C
  Cb         = C + B_bias[k,m]
  S[b,k,:]   = sum_m Cb[b,k,m] U[b,m,k*64:+64]
  out[b,k]   = n2/(n2+1) with n2 = |S|^2

Sharding: data-parallel over batch, 8 batches per core, 8 cores (SPMD).

v4 design (85.4us v2 -> ~69us):
  - ALL matmul operands fp16: 1 cycle/row on the PE (the v2 fp32 routing
    matmuls ran 4 cycles/row = 2 half-speed LOW/HIGH passes).  fp16's 10
    mantissa bits (vs bf16's 8) cut the logit error ~4x, which the
    softmax amplifies exponentially: rel err 3.5e-3 vs v2's 1.8e-2.
  - U is computed in both orientations (oi: [o-part, m-free] for the
    logits matmul + usum; io: [m-part, o-free] for the S matmul), both
    evicted PSUM->SBUF through DVE/ACT alternately (the hard floor:
    fp32 PSUM reads run 1 elem/cycle/engine; bf16-PSUM matmul output is
    TRN3-only, DMA xbar transposes measured 5.6x data amplification,
    and >2 concurrent row-strip matmuls into one PSUM bank lock up the
    device - all three cheaper routes were tried and failed).
  - oi eviction fuses relu + usum via accum_out.
  - the io o-tail (cols 512:640) comes from PE transposes of the
    already-evicted yr_oi c=4 chunk instead of a 5th conv column block:
    transposes may write fp16 PSUM, whose eviction runs in the DVE 2x
    packed mode, and writes stay sequential within the bank.
  - Cb transposes also output fp16 PSUM (2x eviction).
  - input DMAs split across the SP/ACT/GPSIMD queues; gmat + small
    casts on the otherwise idle GPSIMD; final n2 transpose in fp16 to
    drop the fp32 identity operand.
"""

import numpy as np

import concourse.bass as bass
import concourse.mybir as mybir
import concourse.tile as tile
from concourse import bacc
from concourse.bass_utils import run_bass_kernel_spmd

F32 = mybir.dt.float32
F16 = mybir.dt.float16

B_FULL = 64
N = 512          # num timecaps (routing dim m)
DT = 16          # dim timecaps (conv contraction)
K = 10           # classes
D = 64           # dim classes
NO = K * D       # 640 conv output channels
NCORES = 8
BPC = B_FULL // NCORES   # 8 batches per core
N_WARM = 2               # fp16 N=512 warmup matmuls (~3.4us cold -> HAM warm)

NP_F16 = mybir.dt.np(F16)

AF = mybir.ActivationFunctionType
OP = mybir.AluOpType


def _build_program():
    nc = bacc.Bacc("TRN2", target_bir_lowering=False)
    xt_in = nc.declare_dram_parameter("xt", [2, 128, N], F16, isOutput=False)
    w_in = nc.declare_dram_parameter("w", [128, NO], F16, isOutput=False)
    bb_in = nc.declare_dram_parameter("bb", [128, N], F16, isOutput=False)
    id16_in = nc.declare_dram_parameter("id16", [128, 128], F16, isOutput=False)
    gm_in = nc.declare_dram_parameter("gm", [128, 5 * 32], F16, isOutput=False)
    sm_in = nc.declare_dram_parameter("sm", [128, NO], F16, isOutput=False)
    out_d = nc.declare_dram_parameter("out", [BPC, K], F32, isOutput=True)

    with tile.TileContext(nc) as tc:
        with tc.tile_pool(name="const", bufs=1) as cpool:
            w_s = cpool.tile([128, NO], F16, name="w_s", tag="w_s")
            xT = [cpool.tile([128, N], F16, name=f"xT{g}", tag=f"xT{g}")
                  for g in range(2)]
            gmask = cpool.tile([128, 5 * 32], F16, name="gmask", tag="gmask")
            bb_s = cpool.tile([128, N], F16, name="bb_s", tag="bb_s")
            ident16 = cpool.tile([128, 128], F16, name="ident16", tag="ident16")
            smask = cpool.tile([128, NO], F16, name="smask", tag="smask")

            # DMA order = priority order: warmup operand first, then conv
            # operands, then late-phase constants.
            nc.sync.dma_start(w_s[:], w_in[:, :])
            for g in range(2):
                nc.sync.dma_start(xT[g][:], xt_in[g])
            nc.scalar.dma_start(gmask[:], gm_in[:, :])
            nc.scalar.dma_start(bb_s[:], bb_in[:, :])
            nc.gpsimd.dma_start(ident16[:], id16_in[:, :])
            nc.gpsimd.dma_start(smask[:], sm_in[:, :])

            # ---- HAM warmup: fp16 N=512 matmuls on the first-arrived w ----
            with tc.tile_pool(name="ps_warm", bufs=1, space="PSUM") as pw:
                ps_w = pw.tile([128, N], F32, name="ps_w", tag="ps_w")
                for r in range(N_WARM):
                    nc.tensor.matmul(
                        ps_w[:], w_s[0:128, 0:128], w_s[0:128, 0:N],
                        start=(r == 0), stop=(r == N_WARM - 1),
                    )

            with tc.tile_pool(name="data", bufs=1) as dpool:
                yr_oi = [dpool.tile([128, 5 * N], F16, name=f"yr_oi{b}",
                                    tag=f"yr_oi{b}") for b in range(BPC)]
                # io layout: one tile [128, (b, q, 640)] so multi-batch
                # eviction dests are single strided APs
                yr_io_all = dpool.tile([128, BPC * 4 * NO], F16,
                                       name="yr_io_all", tag="yr_io_all")
                yr_io = [yr_io_all[:, b * 4 * NO:(b + 1) * 4 * NO]
                         for b in range(BPC)]
                usum = dpool.tile([128, 5 * BPC], F32, name="usum", tag="usum")
                usum16 = [dpool.tile([128, 5], F16, name=f"usum16_{b}",
                                     tag=f"usum16_{b}") for b in range(BPC)]
                gmat = [dpool.tile([128, 5 * 32], F16, name=f"gmat{b}",
                                   tag=f"gmat{b}") for b in range(BPC)]
                exp_sb = [dpool.tile([128, N], F16, name=f"exp{g}", tag=f"exp{g}")
                          for g in range(2)]
                negmax = [dpool.tile([128, 1], F32, name=f"nm{g}", tag=f"nm{g}")
                          for g in range(2)]
                zsum = [dpool.tile([128, 1], F32, name=f"z{g}", tag=f"z{g}")
                        for g in range(2)]
                rz = [dpool.tile([128, 1], F32, name=f"rz{g}", tag=f"rz{g}")
                      for g in range(2)]
                cb_sb = [dpool.tile([128, N], F16, name=f"cb{g}", tag=f"cb{g}")
                         for g in range(2)]
                ebt_sb = [dpool.tile([128, N], F16, name=f"ebt{g}", tag=f"ebt{g}")
                          for g in range(2)]
                s_sb = [dpool.tile([128, NO], F16, name=f"s{g}", tag=f"s{g}")
                        for g in range(2)]
                n2 = dpool.tile([128, 2], F32, name="n2", tag="n2")
                n216 = dpool.tile([128, 2], F16, name="n216", tag="n216")
                t_c = dpool.tile([2, 128], F32, name="t_c", tag="t_c")
                t_d = dpool.tile([2, 128], F32, name="t_d", tag="t_d")
                outt = dpool.tile([2, 128], F32, name="outt", tag="outt")

                evict_idx = 0

                def evict(dst, src, acc=None):
                    nonlocal evict_idx
                    if evict_idx % 2 == 0:
                        if acc is not None:
                            nc.vector.tensor_scalar(
                                out=dst, in0=src, scalar1=0.0, scalar2=0.0,
                                op0=OP.max, op1=OP.add, accum_out=acc)
                        else:
                            nc.vector.tensor_scalar(
                                out=dst, in0=src, scalar1=0.0, scalar2=None,
                                op0=OP.max)
                    else:
                        nc.scalar.activation(
                            out=dst, in_=src, func=AF.Relu, accum_out=acc)
                    evict_idx += 1

                # PSUM: cv ring 4 + lg 1 + ebt 1 + s 2 = 8 banks
                with tc.tile_pool(name="ps_conv", bufs=4, space="PSUM") as pcv, \
                     tc.tile_pool(name="ps_lg", bufs=1, space="PSUM") as plg, \
                     tc.tile_pool(name="ps_ebt", bufs=1, space="PSUM") as peb, \
                     tc.tile_pool(name="ps_s", bufs=1, space="PSUM") as psp:
                    # ===== phase 1 per g: conv-oi, usum, logits, softmax =====
                    for g in range(2):
                        for c in range(5):
                            tiles = []
                            for j in range(4):
                                ps = pcv.tile([128, N], F32, name="ps_cv",
                                              tag="ps_cv")
                                nc.tensor.matmul(
                                    ps[:],
                                    w_s[32 * j:32 * j + DT + 1,
                                        c * 128:(c + 1) * 128],
                                    xT[g][32 * j:32 * j + DT + 1, :],
                                    start=True, stop=True,
                                    tile_position=(32 * j, 0),
                                )
                                tiles.append(ps)
                            for j in range(4):
                                b = 4 * g + j
                                evict(yr_oi[b][:, c * N:(c + 1) * N],
                                      tiles[j][:],
                                      usum[:, b * 5 + c:b * 5 + c + 1])

                        # gmat = gmask * usum (per batch, fp16 on gpsimd)
                        for j in range(4):
                            b = 4 * g + j
                            nc.gpsimd.tensor_copy(
                                usum16[b][:], usum[:, b * 5:b * 5 + 5])
                            nc.gpsimd.tensor_tensor(
                                out=gmat[b][:].rearrange(
                                    "p (c k) -> p c k", c=5),
                                in0=gmask[:].rearrange(
                                    "p (c k) -> p c k", c=5),
                                in1=usum16[b][:].unsqueeze(2).broadcast_to(
                                    [128, 5, 32]),
                                op=OP.mult,
                            )

                        # logits (col-tiled over j)
                        ps_lg = plg.tile([128, N], F32, name="ps_lg",
                                         tag="ps_lg")
                        for c in range(5):
                            for j in range(4):
                                b = 4 * g + j
                                nc.tensor.matmul(
                                    ps_lg[32 * j:32 * (j + 1), :],
                                    gmat[b][:, c * 32:(c + 1) * 32],
                                    yr_oi[b][:, c * N:(c + 1) * N],
                                    start=(c == 0), stop=(c == 4),
                                    tile_position=(0, 32 * j),
                                    skip_group_check=True,
                                )

                        # softmax -> Cb
                        nc.vector.tensor_reduce(
                            out=negmax[g][:], in_=ps_lg[:],
                            op=OP.max, axis=mybir.AxisListType.X, negate=True,
                        )
                        nc.scalar.activation(
                            out=exp_sb[g][:], in_=ps_lg[:], func=AF.Exp,
                            bias=negmax[g][:], scale=1.0,
                            accum_out=zsum[g][:],
                        )
                        nc.vector.reciprocal(rz[g][:], zsum[g][:])
                        nc.vector.scalar_tensor_tensor(
                            out=cb_sb[g][:], in0=exp_sb[g][:],
                            scalar=rz[g][:], in1=bb_s[:],
                            op0=OP.mult, op1=OP.add,
                        )

                        # Cb transpose (fp16 PSUM -> 2x eviction)
                        ps_eb = peb.tile([128, N], F16, name="ps_eb",
                                         tag="ps_eb")
                        for q in range(4):
                            nc.tensor.transpose(
                                ps_eb[:, q * 128:(q + 1) * 128],
                                cb_sb[g][:, q * 128:(q + 1) * 128],
                                ident16[:],
                            )
                        nc.vector.tensor_copy(ebt_sb[g][:], ps_eb[:])

                    # ===== phase 2 per g: conv-io, S, squash tail =====
                    # yr_io layout per batch: [128(m of chunk q), (q, 640 o)]
                    for g in range(2):
                        ps_s = psp.tile([128, NO], F32, name="ps_s", tag="ps_s")
                        for q in range(4):
                            tiles = []
                            for j in range(4):
                                ps = pcv.tile([128, N], F32, name="ps_io",
                                              tag="ps_cv")
                                nc.tensor.matmul(
                                    ps[:],
                                    xT[g][32 * j:32 * j + DT + 1,
                                          q * 128:(q + 1) * 128],
                                    w_s[32 * j:32 * j + DT + 1, 0:512],
                                    start=True, stop=True,
                                    tile_position=(32 * j, 0),
                                )
                                tiles.append(ps)
                            for j in range(4):
                                b = 4 * g + j
                                evict(yr_io[b][:, q * NO:q * NO + 512],
                                      tiles[j][:])
                            # o-tail (cols 512:640) via PE transposes of the
                            # already-evicted yr_oi c=4 chunk: sequential
                            # writes into one fp16 PSUM bank, 2x eviction
                            ps_tl = peb.tile([128, N], F16, name="ps_tl",
                                             tag="ps_eb")
                            for j in range(4):
                                b = 4 * g + j
                                nc.tensor.transpose(
                                    ps_tl[:, j * 128:(j + 1) * 128],
                                    yr_oi[b][:, 4 * N + q * 128:
                                             4 * N + (q + 1) * 128],
                                    ident16[:],
                                )
                            evict(yr_io_all[:].rearrange(
                                      "p (b q o) -> p b q o", b=BPC, q=4)
                                  [:, 4 * g:4 * g + 4, q, 512:NO],
                                  ps_tl[:])
                            # S matmuls for this q (col-tiled over j)
                            for j in range(4):
                                b = 4 * g + j
                                nc.tensor.matmul(
                                    ps_s[32 * j:32 * (j + 1), 0:512],
                                    ebt_sb[g][:, q * 128 + 32 * j:
                                              q * 128 + 32 * (j + 1)],
                                    yr_io[b][:, q * NO:q * NO + 512],
                                    start=(q == 0), stop=(q == 3),
                                    tile_position=(0, 32 * j),
                                    skip_group_check=True,
                                )
                                nc.tensor.matmul(
                                    ps_s[32 * j:32 * (j + 1), 512:NO],
                                    ebt_sb[g][:, q * 128 + 32 * j:
                                              q * 128 + 32 * (j + 1)],
                                    yr_io[b][:, q * NO + 512:q * NO + NO],
                                    start=(q == 0), stop=(q == 3),
                                    tile_position=(0, 32 * j),
                                    skip_group_check=True,
                                )

                        # masked squash tail
                        nc.vector.tensor_tensor(
                            out=s_sb[g][:], in0=ps_s[:], in1=smask[:],
                            op=OP.mult,
                        )
                        nc.vector.scalar_tensor_tensor(
                            out=s_sb[g][:], in0=s_sb[g][:],
                            scalar=1.0, in1=s_sb[g][:],
                            op0=OP.mult, op1=OP.mult,
                            accum_out=n2[:, g:g + 1],
                        )

                # transpose n2 -> [2(g), 128(32j+k)]; out = 1 - 1/(n2+1)
                with tc.tile_pool(name="ps_t2", bufs=1, space="PSUM") as pt2:
                    tr2 = pt2.tile([2, 128], F16, name="ps_tr2", tag="ps_tr2")
                    nc.gpsimd.tensor_copy(n216[:], n2[:])
                    nc.tensor.transpose(tr2[:], n216[:], ident16[:])
                    nc.vector.tensor_scalar(
                        out=t_c[:], in0=tr2[:], scalar1=1.0, scalar2=None,
                        op0=OP.add)
                    nc.vector.reciprocal(t_d[:], t_c[:])
                    nc.vector.tensor_scalar(
                        out=outt[:], in0=t_d[:], scalar1=-1.0, scalar2=1.0,
                        op0=OP.mult, op1=OP.add)
                    nc.sync.dma_start(
                        out_d.rearrange("(g j) k -> g j k", g=2),
                        outt[:].rearrange("g (j k) -> g j k", j=4)[:, :, 0:K],
                    )
    nc.compile()
    return nc


_PROGRAM_CACHE = None


def _get_program():
    global _PROGRAM_CACHE
    if _PROGRAM_CACHE is None:
        _PROGRAM_CACHE = _build_program()
    return _PROGRAM_CACHE


def make_in_maps(timecaps, conv_w, conv_b, B_bias):
    """Host-side prep: per-core transposed/padded operand arrays."""
    timecaps = np.ascontiguousarray(np.asarray(timecaps, dtype=np.float32))
    conv_w = np.asarray(conv_w, dtype=np.float32)
    conv_b = np.asarray(conv_b, dtype=np.float32)
    B_bias = np.asarray(B_bias, dtype=np.float32)

    w4 = np.zeros((128, NO), np.float32)
    bb = np.zeros((128, N), np.float32)
    gm = np.zeros((128, 5 * 32), np.float32)
    sm = np.zeros((128, NO), np.float32)
    for j in range(4):
        w4[32 * j:32 * j + DT] = conv_w[:, 0, :]
        w4[32 * j + DT] = conv_b
        bb[32 * j:32 * j + K] = B_bias[:, 0, :]
        sm_rows = sm[32 * j:32 * j + K]
        for k in range(K):
            sm_rows[k, D * k:D * (k + 1)] = 1.0
    for c in range(5):
        for p in range(128):
            gm[p, c * 32 + (c * 128 + p) // D] = 0.25
    ident = np.eye(128, dtype=np.float32)

    # xT per core: [2, 128, 512], rows 32j+t = x[4g+j, :, t], row 32j+16 = 1
    xs = timecaps.reshape(NCORES, 2, 4, N, DT).transpose(0, 1, 2, 4, 3)
    in_maps = []
    shared = {
        "w": w4.astype(NP_F16),
        "bb": bb.astype(NP_F16),
        "id16": ident.astype(NP_F16),
        "gm": gm.astype(NP_F16),
        "sm": sm.astype(NP_F16),
    }
    for core in range(NCORES):
        xt = np.zeros((2, 128, N), np.float32)
        for j in range(4):
            xt[:, 32 * j:32 * j + DT] = xs[core, :, j]
            xt[:, 32 * j + DT] = 1.0
        in_maps.append({"xt": xt.astype(NP_F16), **shared})
    return in_maps


def kernel(timecaps, conv_w, conv_b, B_bias):
    nc = _get_program()
    in_maps = make_in_maps(timecaps, conv_w, conv_b, B_bias)
    res = run_bass_kernel_spmd(nc, in_maps, list(range(NCORES)))
    out = np.concatenate([res.results[i]["out"] for i in range(NCORES)], axis=0)
    return out.reshape(B_FULL, K, 1).astype(np.float32)


if __name__ == "__main__":
    rng = np.random.default_rng(0)
    ins = {
        "timecaps": rng.standard_normal((B_FULL, N, DT), dtype=np.float32),
        "conv_w": (rng.standard_normal((DT, 1, NO), dtype=np.float32) * 0.05),
        "conv_b": np.zeros((NO,), dtype=np.float32),
        "B_bias": (rng.standard_normal((K, 1, N), dtype=np.float32) * 0.05),
    }
    print(kernel(**ins)[:2, :, 0])
